# revision 43
# baseline (speedup 1.0000x reference)
"""Paged sliding-window decode attention (GQA + sinks) on 8 TRN2 NeuronCores.

Sharding: tensor-parallel over the 8 KV heads -- core g handles KV head g
(and its 4 grouped query heads) for ALL 8 sequences.  This is perfectly
load-balanced regardless of per-sequence context lengths.

Host side (not on the device-critical path): for each sequence we slice the
valid sliding-window region of the paged KV cache (<= 1024 contiguous
positions; block tables are walked generally), splice in the newly-written
k/v token, convert to bf16, and pack everything into ONE "ring" array laid
out in exact device-consumption order with two seqs of QK lookahead:
  ring [128, cols] = [qt | K_s0 | K_s1 | K_s2 | V_s0 | K_s3 | V_s1 | ...
                      | V_s6 | V_s7]
    K_b [128=d, nch_b*128]    K transposed, zero-padded to 128-token chunks
    V_b [128=t, nch_b*VCOLS]  V in 128-token chunks + a ones-column that
                              accumulates the softmax denominator inside PV
    qt  [128=d, 32=(b,h)]     queries grouped (seq, local head)
  sks [GQ, B]                 per-head attention-sink logits

Device side per core: a SINGLE FIFO DMA stream on sync/HWDGE (one queue
already saturates the practical per-core HBM rate, and a single FIFO gives
deterministic arrival order for the in-order PE; multi-queue splits the
same bandwidth and adds arrival skew that shows up as multi-us PE stalls).
Pieces are triangle-graded: fine at both ends for low fill/drain latency,
wide in the middle where big per-partition descriptors sustain the highest
SDMA rate.  QK^T matmuls run in token-partition orientation (no transposes
anywhere), one exp() per sequence (softmax without max-subtraction --
scaled scores are ~N(0,1), so exp is safe in f32 and mathematically
identical), PV matmuls accumulate numerator + denominator, reciprocal
multiply into a staging tile, and a split DMA out rides gpsimd/SWDGE.
The PE schedule carries two sequences of QK lookahead so each exp's
cross-engine round-trip hides under the next seqs' QK work.

After tile sem-assignment, _elide_matmul_ticks strips the per-matmul tick
increments nobody waits on (the per-proc max-tick dependency is always the
last matmul of a chunk group) and remaps the surviving wait values; the
de-sem'd matmuls then HW-decode at ~2 ns dispatch instead of ~128 ns,
roughly halving the PE stream time.
"""

import os
import numpy as np
from contextlib import ExitStack

B = 8
H = 32
KVH = 8
GQ = H // KVH          # 4 query heads per kv head
D = 128
BS = 16                # tokens per cache block
MAX_CTX = 4096
WIN = 1024
SCALE = 0.08838834764831845
CHUNK = 128            # PV contraction tile (token partition dim)
VCOLS = 129            # 128 v dims + 1 ones-column
NRINGS = 1             # single sync input stream: one queue already reaches
                       # the practical per-core HBM rate (~320-400 B/ns when
                       # running solo), and a single FIFO gives deterministic
                       # arrival order for the in-order PE -- multi-queue
                       # splits the same bandwidth and adds arrival skew that
                       # showed up as multi-us PE stalls

KV_BF16 = os.environ.get("KERNEL_KV_BF16", "1") == "1"
FAST_TAIL = os.environ.get("KERNEL_FAST_TAIL", "1") == "1"
ELIDE_TICKS = os.environ.get("KERNEL_ELIDE_TICKS", "1") == "1"

# Matmuls whose completion no cross-engine consumer waits on directly (tile
# waits use the per-proc MAX tick of an instruction's dependencies, which is
# always the final matmul of each chunk group).  Stripping their sem incs
# after tile's sem-assignment lets the PE HW-decode them (~2ns dispatch)
# instead of the SW-decode + sem-bookkeeping path (~128ns per instruction).
_NO_TICK_NAMES: set[str] = set()


def _elide_matmul_ticks(nc):
    """Strip PE tick-sem incs from matmuls in _NO_TICK_NAMES; remap every
    wait value on that sem to count only the surviving incs.  Any matmul
    whose ordinal a wait actually references is force-kept, so every wait
    still fires exactly when its true dependency completes."""
    import concourse.mybir as mybir

    mms, all_insts, sem_id = [], [], None
    for func in nc.m.functions:
        for block in func.blocks:
            for inst in block.instructions:
                all_insts.append(inst)
                if isinstance(inst, mybir.InstMatmult):
                    mms.append(inst)
                    si = inst.sync_info
                    if sem_id is None and si and si.on_update:
                        sem_id = si.on_update[0].id
    if sem_id is None:
        return
    for m in mms:  # the pass assumes one inc-by-1 on the tick sem per matmul
        si = m.sync_info
        assert si and len(si.on_update) == 1 and si.on_update[0].id == sem_id \
            and si.on_update[0].update_value == 1, m.name

    wait_vals = set()
    for inst in all_insts:
        si = inst.sync_info
        if not si:
            continue
        for w in si.on_wait:
            if w.id == sem_id:
                assert w.wait_mode == "sem-ge-imm", (inst.name, w.wait_mode)
                wait_vals.add(w.wait_value)

    keep = [m.name not in _NO_TICK_NAMES for m in mms]
    for v in wait_vals:
        keep[v - 1] = True  # ">= v" references matmul #v (1-based)
    pref, c = [], 0
    for k in keep:
        c += k
        pref.append(c)
    for i, m in enumerate(mms):
        if not keep[i]:
            m.sync_info.on_update = []
    for inst in all_insts:
        si = inst.sync_info
        if not si or not si.on_wait:
            continue
        if any(w.id == sem_id for w in si.on_wait):
            for w in si.on_wait:
                if w.id == sem_id:
                    w.wait_value = pref[w.wait_value - 1]


def _host_shards(q, k, v, k_cache, v_cache, sinks, block_tables, context_lens,
                 slot_mapping):
    """Slice/lay out the full inputs into per-core input arrays."""
    ctx = np.asarray(context_lens, dtype=np.int64)
    bt = np.asarray(block_tables, dtype=np.int64)
    n = np.minimum(ctx, WIN)                      # window sizes
    start = ctx - n
    offs = np.zeros(B + 1, np.int64)
    offs[1:] = np.cumsum(n)
    Ttot = int(offs[-1])
    nch = (n + CHUNK - 1) // CHUNK
    choffs = np.zeros(B + 1, np.int64)
    choffs[1:] = np.cumsum(nch)
    NCH = int(choffs[-1])

    kq = np.asarray(k, np.float32).reshape(B, KVH, D)
    vq = np.asarray(v, np.float32).reshape(B, KVH, D)

    # gather windowed KV rows (general block-table walk) + splice new token
    kwin = np.empty((Ttot, KVH, D), np.float32)
    vwin = np.empty((Ttot, KVH, D), np.float32)
    for b in range(B):
        pos = np.arange(start[b], ctx[b])
        rows = bt[b, pos // BS] * BS + pos % BS
        kwin[offs[b]:offs[b + 1]] = k_cache[rows]
        vwin[offs[b]:offs[b + 1]] = v_cache[rows]
        # new token sits at position ctx-1 == last row of the window
        kwin[offs[b + 1] - 1] = kq[b]
        vwin[offs[b + 1] - 1] = vq[b]

    import ml_dtypes
    kv_np = np.dtype(ml_dtypes.bfloat16) if KV_BF16 else np.dtype("float32")
    v_np = kv_np   # fp8 for the PV side measured 3% rel err -- too lossy

    # per-head transposed K, zero-padded per segment to 128-token chunks so
    # QK matmuls always write full psum partitions: [KVH, D, NCH*CHUNK]
    kt_raw = np.ascontiguousarray(kwin.transpose(1, 2, 0))  # [KVH, D, Ttot]
    kt_all = np.zeros((KVH, D, NCH * CHUNK), np.float32)
    for b in range(B):
        k0 = int(choffs[b]) * CHUNK
        kt_all[:, :, k0:k0 + int(n[b])] = kt_raw[:, :, offs[b]:offs[b + 1]]

    # per-head chunked V (+ ones column): [KVH, CHUNK, NCH*VCOLS]
    vch = nch.copy()
    vchoffs = np.zeros(B + 1, np.int64)
    vchoffs[1:] = np.cumsum(vch)
    vt_all = np.zeros((KVH, CHUNK, int(vchoffs[-1]) * VCOLS), np.float32)
    for b in range(B):
        for c in range(int(nch[b])):
            w = int(min(CHUNK, n[b] - c * CHUNK))
            base = int((vchoffs[b] + c) * VCOLS)
            seg = vwin[offs[b] + c * CHUNK: offs[b] + c * CHUNK + w]  # [w,KVH,D]
            vt_all[:, 0:w, base:base + D] = seg.transpose(1, 0, 2)
            vt_all[:, 0:w, base + D] = 1.0

    qr = np.asarray(q, np.float32).reshape(B, KVH, GQ, D)
    qt_all = np.ascontiguousarray(qr.transpose(1, 3, 0, 2).reshape(KVH, D, B * GQ))

    sk = np.asarray(sinks, np.float32).reshape(KVH, GQ)

    # Ring blobs in exact device-consumption order, one ring per DMA-issuing
    # engine (sync / gpsimd / scalar).  Ring r's columns: [qt (r=0) | K_s0 |
    # K_s1 | V_s0 | K_s2 | V_s1 | ... | V_slast].  Each stream delivers this
    # sequentially, so the software-pipelined PE order never waits on
    # out-of-order data.  Seqs are LPT-balanced across rings by chunk count
    # (round-robin SDMA arbitration gives each non-empty queue an equal
    # bandwidth share, so ring byte balance decides the stream end time).
    NR = NRINGS
    cap = (B + NR - 1) // NR
    order = sorted(range(B), key=lambda b: -int(nch[b]))
    loads = [0] * NR
    # sync's HWDGE queue consistently sustains a somewhat higher rate than
    # gpsimd's SWDGE queue; bias the byte split accordingly
    weight = [1.15, 1.0, 1.0][:NR]
    ring_seqs = [[] for _ in range(NR)]
    for b in order:
        r = min((i for i in range(NR) if len(ring_seqs[i]) < cap),
                key=lambda i: (loads[i] + int(nch[b])) / weight[i])
        ring_seqs[r].append(b)
        loads[r] += int(nch[b])

    # ring -> engine: ring0=sync (carries qt, feeds the first PE work),
    # ring1=gpsimd (SWDGE has its own sem lanes + spare capacity: give it
    # the heaviest ring), ring2=scalar (lightest, so its DGE issues clear
    # the queue before the exps).  Only 8 HWDGE sem lanes exist shared by
    # sync+scalar; a 9th HWDGE dma_start would stall its whole engine
    # waiting on an in-flight lane, so each HWDGE ring gets <=4 pieces.
    by_load = sorted(range(NR), key=lambda r: -loads[r])
    if NR == 3:
        perm = [by_load[1], by_load[0], by_load[2]]   # mid, heavy, light
        ring_seqs = [ring_seqs[r] for r in perm]
        loads = [loads[r] for r in perm]

    kof, vof = {}, {}
    ring_cols = []
    cuts = []        # per-ring piece cut points
    for r, S in enumerate(ring_seqs):
        o = B * GQ if r == 0 else 0
        small = o + min(2, int(nch[S[0]])) * CHUNK   # qt + 2 chunks of K_s0
        # segments in consumption order, two seqs of lookahead so the PE has
        # K_{i+1}, K_{i+2} QK work in hand while exp(s_i) round-trips through
        # the scalar engine: K_s0, K_s1, K_s2, V_s0, K_s3, V_s1, ...
        LOOK = 3
        segs = [("k", S[i]) for i in range(min(LOOK, len(S)))]
        for i in range(LOOK, len(S)):
            segs.append(("k", S[i]))
            segs.append(("v", S[i - LOOK]))
        for i in range(max(0, len(S) - LOOK), len(S)):
            segs.append(("v", S[i]))
        bounds = []                                   # allowed cut points
        for kind, b in segs:
            if kind == "k":
                kof[b] = (r, o)
                o += int(nch[b]) * CHUNK
            else:
                vof[b] = (r, o)
                o += int(nch[b]) * VCOLS
            bounds.append(o)
        _, ovl = vof[S[-1]]
        # tiny last piece (2 chunks of V_last) so little PV trails the stream
        tail = ovl + max(0, int(nch[S[-1]]) - 2) * VCOLS
        cc = [0, small, tail, o]
        # triangle-graded piece widths: fine at both ends (short consumer
        # stalls while the PE ramps, and prompt completion sems for the
        # closing PVs), wide in the middle (bigger per-partition descriptors
        # sustain a higher SDMA rate through the bulk of the stream)
        last_cut = small
        for bnd in bounds:
            frac = bnd / max(tail, 1)
            tgt = 900 + int(2200 * min(frac, 1.0 - frac))
            if bnd - last_cut >= tgt and bnd < tail:
                cc.append(bnd)
                last_cut = bnd
        ring_cols.append(o)
        # drop degenerate pieces (possible with tiny contexts)
        cuts.append(sorted(set(c for c in cc if c <= o)))

    in_maps = [dict() for _ in range(KVH)]
    for g in range(KVH):
        for r in range(NR):
            blob = np.empty((D, ring_cols[r]), np.float32)
            if r == 0:
                blob[:, 0:B * GQ] = qt_all[g]
            for b in ring_seqs[r]:
                _, o = kof[b]
                w = int(nch[b]) * CHUNK
                k0 = int(choffs[b]) * CHUNK
                blob[:, o:o + w] = kt_all[g][:, k0:k0 + w]
                _, o = vof[b]
                w = int(nch[b]) * VCOLS
                blob[:, o:o + w] = vt_all[g][:, choffs[b] * VCOLS:
                                             choffs[b] * VCOLS + w]
            in_maps[g][f"ring{r}"] = np.ascontiguousarray(blob.astype(kv_np))
        in_maps[g]["sks"] = np.ascontiguousarray(
            np.tile(sk[g][:, None], (1, B)).astype(np.float32))
    # output (and o_cat) columns in FINISH order = round-robin across rings
    finish = []
    for i in range(cap):
        for S in ring_seqs:
            if i < len(S):
                finish.append(S[i])
    meta = dict(n=n, offs=offs, Ttot=Ttot, nch=nch, choffs=choffs, NCH=NCH,
                ring_cols=ring_cols, cuts=cuts, kof=kof, vof=vof,
                ring_seqs=ring_seqs, finish=finish)
    return in_maps, meta


def _build_graph(meta):
    import concourse.bass as bass
    import concourse.tile as tile
    from concourse import bacc, mybir

    n, nch = meta["n"], meta["nch"]
    ring_cols, cuts = meta["ring_cols"], meta["cuts"]
    kof, vof = meta["kof"], meta["vof"]

    f32 = mybir.dt.float32
    kdt = mybir.dt.bfloat16 if KV_BF16 else f32

    if ELIDE_TICKS:
        _NO_TICK_NAMES.clear()

    nc = bacc.Bacc("TRN2", target_bir_lowering=False, debug=False,
                   num_devices=KVH)
    ring_d = [nc.dram_tensor(f"ring{r}", [D, ring_cols[r]], kdt,
                             kind="ExternalInput") for r in range(NRINGS)]
    sks_d = nc.dram_tensor("sks", [GQ, B], f32, kind="ExternalInput")
    out_d = nc.dram_tensor("out", [GQ, B * D], f32, kind="ExternalOutput")

    tc_cls = tile.TileContext
    if FAST_TAIL:
        class _FastTailTileContext(tile.TileContext):
            # Keep the drain (sync waits for every sem's final value, which
            # covers the output DMA) and one all-engine barrier; skip the
            # per-sem clear + second barrier.  (Barriering only a subset of
            # engines was measured NOT to move the walrus sem-sweep epilogue
            # earlier -- walrus gates the sweeps on its own end-of-program
            # rendezvous.)  Safe as long as each execute runs a freshly-
            # loaded NEFF (bass2jax builds a new executable per kernel()
            # call, and NEFF load resets semaphore state).
            def _drain_and_barrier(self, tick_clock, wait_clock):
                from concourse.tile import ScopedClock
                drain_inst = self.nc.sync.drain()
                wait_clock.add_sem_waits(
                    drain_inst.ins, ScopedClock({None: tick_clock.global_clock}))
                self.nc.all_engine_barrier()
                popped = self.nc._tile_sem_poison_stack.pop()
                assert popped is self._sem_poison
        tc_cls = _FastTailTileContext

    pam = os.environ.get("KERNEL_POOL_MODE", "stack")
    with tc_cls(nc, pool_alloc_mode=pam) as tc, ExitStack() as es:
        kv_pool = es.enter_context(tc.tile_pool(name="kv", bufs=1))
        s_pool = es.enter_context(tc.tile_pool(name="sT", bufs=4, space="PSUM"))
        o_pool = es.enter_context(tc.tile_pool(name="o", bufs=1, space="PSUM"))
        e_pool = es.enter_context(tc.tile_pool(name="eT", bufs=8))
        w_pool = es.enter_context(tc.tile_pool(name="work", bufs=1))

        rings = [kv_pool.tile([D, ring_cols[r]], kdt, tag=f"ring{r}",
                              name=f"ringt{r}") for r in range(NRINGS)]
        # three FIFO DMA streams in consumption order: sync (HWDGE ring A)
        # carries ring0, gpsimd (SWDGE) ring1, scalar (HWDGE ring B) ring2.
        # scalar also runs the exps, so only its first two pieces are issued
        # up front; the rest are issued after the first QK round so the
        # early exps aren't stuck behind a full DGE ring.
        sks_sb = w_pool.tile([GQ, B], f32, tag="sks")
        nc.gpsimd.dma_start(out=sks_sb[:], in_=sks_d[:, :])
        ring_engs = [nc.sync, nc.gpsimd, nc.scalar][:NRINGS]
        for r, eng in enumerate(ring_engs):
            for lo, hi in zip(cuts[r][:-1], cuts[r][1:]):
                eng.dma_start(out=rings[r][:, lo:hi], in_=ring_d[r][:, lo:hi])

        esk = w_pool.tile([GQ, B], f32, tag="esk")
        nc.scalar.activation(esk[:], sks_sb[:],
                             mybir.ActivationFunctionType.Exp)
        qt_slice = rings[0][:, 0:B * GQ]
        kts = kof
        vts = vof

        # PSUM accumulation tiles: pairs of in-ring-consecutive seqs.  A
        # ring's PV chains close in order, so the earlier seq of a pair is
        # done before its partner's chain opens -- no foreign start=True can
        # clobber an open accumulation group sharing a tile (each ring's
        # split tail seq sits last in its pair).  A pair is <=258 f32 cols,
        # under the 512-col PSUM bank limit.
        o_groups = []
        for r, S in enumerate(meta["ring_seqs"]):
            for i in range(0, len(S), 2):
                o_groups.append(S[i:i + 2])
        o_tiles = [o_pool.tile([GQ, len(gseqs) * VCOLS], f32,
                               tag=f"og{gi}", name=f"ogt{gi}")
                   for gi, gseqs in enumerate(o_groups)]
        grp_of = {b: (gi, j) for gi, gseqs in enumerate(o_groups)
                  for j, b in enumerate(gseqs)}
        o_cat = w_pool.tile([GQ, B * D], f32, tag="ocat")
        den = w_pool.tile([GQ, B], f32, tag="den")
        rec = w_pool.tile([GQ, B], f32, tag="rec")
        eTs = {}
        # o_cat columns in FINISH order so the output DMA splits into two
        # contiguous pieces (early seqs / trailing seqs); host un-permutes
        ring_seqs = meta["ring_seqs"]
        finish = meta["finish"]
        pos = {b: i for i, b in enumerate(finish)}

        sTs = {}

        def emit_qk(b, c_lo=0, c_hi=None):
            ncb = int(nch[b])
            if c_hi is None:
                c_hi = ncb
            c_lo = min(c_lo, ncb)
            pk, ok = kts[b]
            if b not in sTs:
                sTs[b] = s_pool.tile([CHUNK, ncb * GQ], f32, tag="sT",
                                     name=f"sT{b}")
            sT = sTs[b]
            for c in range(c_lo, c_hi):
                mm = nc.tensor.matmul(
                    sT[:, GQ * c:GQ * (c + 1)],
                    rings[pk][:, ok + c * CHUNK:ok + (c + 1) * CHUNK],
                    qt_slice[:, GQ * b:GQ * (b + 1)],
                    start=True, stop=True)
                if ELIDE_TICKS and c != ncb - 1:
                    _NO_TICK_NAMES.add(mm.ins.name)
            if c_hi < ncb:
                return
            eT = e_pool.tile([CHUNK, ncb * GQ], kdt, tag="eT", name=f"eT{b}")
            nc.scalar.activation(eT[:], sT[:],
                                 mybir.ActivationFunctionType.Exp, scale=SCALE)
            eTs[b] = eT

        def emit_pv(b, c_lo=0, c_hi=None):
            nb, ncb = int(n[b]), int(nch[b])
            if c_hi is None:
                c_hi = ncb
            c_lo = min(c_lo, ncb)
            pv, ov = vts[b]
            eT = eTs[b]
            gi, j = grp_of[b]
            o_ps = o_tiles[gi][:, j * VCOLS:(j + 1) * VCOLS]
            for c in range(c_lo, c_hi):
                w = min(CHUNK, nb - c * CHUNK)
                mm = nc.tensor.matmul(
                    o_ps,
                    eT[0:w, GQ * c:GQ * (c + 1)],
                    rings[pv][0:w, ov + c * VCOLS:ov + (c + 1) * VCOLS],
                    start=(c == 0), stop=(c == ncb - 1),
                    skip_group_check=True)
                if ELIDE_TICKS and c != ncb - 1:
                    _NO_TICK_NAMES.add(mm.ins.name)
            if c_hi < ncb:
                return
            # per-seq epilogue: denom = sum(e) + exp(sink); out = num/denom
            nc.vector.tensor_add(den[:, b:b + 1], o_ps[:, D:D + 1],
                                 esk[:, b:b + 1])
            nc.vector.reciprocal(rec[:, b:b + 1], den[:, b:b + 1])
            nc.vector.tensor_scalar_mul(
                o_cat[:, pos[b] * D:(pos[b] + 1) * D],
                o_ps[:, 0:D], rec[:, b:b + 1])

        # software-pipelined PE order matching the rings' delivery order
        # (per ring: K_s0, K_s1, V_s0, K_s2, V_s1, ..., V_slast), rings
        # interleaved round-robin.  Each ring's final PV is split so only
        # ~2 chunks of PV work trail the end of its DMA stream.
        h0 = {S[0]: (2 if int(nch[S[0]]) > 2 else 0) for S in ring_seqs}
        # emit tail remainders in ring-stream-end order (smallest ring first)
        ring_load = [sum(int(nch[b]) for b in S) for S in ring_seqs]
        tail_order = sorted(range(len(ring_seqs)), key=lambda r: ring_load[r])
        tails = [(ring_seqs[r][-1], max(0, int(nch[ring_seqs[r][-1]]) - 2))
                 for r in tail_order]
        tail_of = dict(tails)
        LOOK = 3     # seqs of QK lookahead hiding the exp round-trip
        per_ring = []
        for S in ring_seqs:
            ring_steps = [("qkp", S[0]), ("qkr", S[0])]
            for i in range(1, min(LOOK, len(S))):
                ring_steps.append(("qk", S[i]))
            for i in range(LOOK, len(S)):
                ring_steps.append(("qk", S[i]))
                ring_steps.append(("pv", S[i - LOOK]))
            for i in range(max(0, len(S) - LOOK), len(S)):
                ring_steps.append(("pv", S[i]))
            per_ring.append(ring_steps)
        merged = []
        for rnd in range(max(len(rs) for rs in per_ring)):
            for rs in per_ring:
                if rnd < len(rs):
                    merged.append(rs[rnd])
        for op, b in merged:
            if op == "qkp":
                emit_qk(b, 0, h0[b])
            elif op == "qkr":
                emit_qk(b, h0[b], None)
            elif op == "qk":
                emit_qk(b)
            elif b in tail_of:
                emit_pv(b, 0, tail_of[b])
            else:
                emit_pv(b)
        for b, t in tails:
            emit_pv(b, t, None)

        # non-tail seqs' output streams out while the ring tails finish;
        # gpsimd's SWDGE lanes are free by now (HWDGE lanes may not be)
        split = (B - len(tails)) * D
        nc.gpsimd.dma_start(out=out_d[:, 0:split], in_=o_cat[:, 0:split])
        nc.gpsimd.dma_start(out=out_d[:, split:], in_=o_cat[:, split:])

    if ELIDE_TICKS and _NO_TICK_NAMES:
        _elide_matmul_ticks(nc)
    nc.compile()
    return nc


def _patch_walrus_flags():
    extra = os.environ.get("KERNEL_WALRUS_EXTRA", "")
    if not extra:
        return
    import concourse.bass_utils as bu
    if getattr(bu, "_kernel_walrus_patched", None) == extra:
        return
    orig_rc = bu.run_command

    def rc(argv, **kw):
        if argv and "walrus" in str(argv[0]):
            argv = list(argv) + extra.split(":")
        return orig_rc(argv, **kw)

    bu.run_command = rc
    bu._kernel_walrus_patched = extra


def _run(inputs, trace=False, trace_kwargs=None):
    from concourse.bass_utils import run_bass_kernel_spmd
    _patch_walrus_flags()

    in_maps, meta = _host_shards(**inputs)
    nc = _build_graph(meta)
    kw = {}
    if trace_kwargs:
        kw.update(trace_kwargs)
    res = run_bass_kernel_spmd(nc, in_maps, core_ids=list(range(KVH)),
                               trace=trace, **kw)
    finish = meta["finish"]
    out = np.empty((B, H, D), np.float32)
    for g in range(KVH):
        og = np.asarray(res.results[g]["out"], np.float32)  # [GQ, B*D]
        o3 = og.reshape(GQ, B, D).transpose(1, 0, 2)        # [finish_pos,GQ,D]
        out[finish, g * GQ:(g + 1) * GQ, :] = o3
    return out.reshape(B, H * D), res


def kernel(**inputs):
    out, _ = _run(inputs, trace=False)
    return out

